# revision 8
# baseline (speedup 1.0000x reference)
"""AdaptiveSpectralDNA Trainium2 kernel: 8-core data-parallel SIREN MLP.

Layout: feature-major activations ([features on partitions, rows on free dim])
so no transposes are ever needed.  Matmuls run in float32r (full PE rate at
fp32 storage).  sin(omega*(Wx+b)) is computed as sin2pi(frac) where
frac = t2 - round(t2), t2 = (omega/2pi)*(Wx+b); round() via the fp32
magic-number trick; sin2pi is an ACT LUT entry reached by patching the BIR
(bass has no enum for it) and lives in the same table set as sigmoid.
"""
import os
import numpy as np

N = 524288
COORD_DIM = 4
HIDDEN = 256
NHL = 4
NC = 8
NCORE = N // NC          # 65536 rows per core
R = 512                  # rows per tile (one PSUM bank of fp32)
T = NCORE // R           # 128 tiles
MAGIC = float(1.5 * 2**23)
INV2PI = float(1.0 / (2.0 * np.pi))
MIN_O, MAX_O = 10.0, 100.0

_CACHE = {}


def _build():
    import concourse.bass as bass
    import concourse.mybir as mybir
    from concourse.tile import TileContext

    F32 = mybir.dt.float32
    F32R = mybir.dt.float32r
    A = mybir.ActivationFunctionType
    ALU = mybir.AluOpType

    nc = bass.Bass()
    coordsT = nc.declare_dram_parameter("coordsT", [4, NCORE], F32R, isOutput=False)
    coordsTF = nc.declare_dram_parameter("coordsTF", [4, NCORE], F32, isOutput=False)
    WH_e = nc.declare_dram_parameter("WH", [128, 16 * 128], F32R, isOutput=False)
    W0_e = nc.declare_dram_parameter("W0", [4, 256], F32, isOutput=False)
    OW1_e = nc.declare_dram_parameter("OW1", [4, 64], F32R, isOutput=False)
    OW2R_e = nc.declare_dram_parameter("OW2R", [64, 128], F32R, isOutput=False)
    WF_e = nc.declare_dram_parameter("WF", [128, 2], F32R, isOutput=False)
    BIAS0_e = nc.declare_dram_parameter("BIAS0", [1, 256], F32, isOutput=False)
    BIASH_e = nc.declare_dram_parameter("BIASH", [1, 8 * 128], F32R, isOutput=False)
    OB1_e = nc.declare_dram_parameter("OB1", [64, 1], F32, isOutput=False)
    OB2C_e = nc.declare_dram_parameter("OB2C", [128, 1], F32, isOutput=False)
    MAGC_e = nc.declare_dram_parameter("MAGC", [128, 1], F32, isOutput=False)
    out_e = nc.declare_dram_parameter("out", [NCORE], F32, isOutput=True)

    with TileContext(nc) as tc:
        with (
            tc.tile_pool(name="wpool", bufs=1) as wpool,
            tc.tile_pool(name="cpool", bufs=3) as cpool,
            tc.tile_pool(name="work", bufs=3) as work,
            tc.tile_pool(name="ypool", bufs=3) as ypool,
            tc.tile_pool(name="opool", bufs=3) as opool,
            tc.tile_pool(name="pz", bufs=4, space="PSUM") as pz,
        ):
            WH = wpool.tile([128, 16 * 128], F32R)
            W0 = wpool.tile([4, 256], F32)
            OW1 = wpool.tile([4, 64], F32R)
            OW2R = wpool.tile([64, 128], F32R)
            WF = wpool.tile([128, 2], F32R)
            BIAS0 = wpool.tile([1, 256], F32)
            BIASH = wpool.tile([1, 8 * 128], F32R)
            OB1 = wpool.tile([64, 1], F32)
            OB2C = wpool.tile([128, 1], F32)
            MAGC = wpool.tile([128, 1], F32)
            nc.sync.dma_start(out=WH[:], in_=WH_e[:])
            nc.sync.dma_start(out=W0[:], in_=W0_e[:])
            nc.sync.dma_start(out=OW1[:], in_=OW1_e[:])
            nc.sync.dma_start(out=OW2R[:], in_=OW2R_e[:])
            nc.sync.dma_start(out=WF[:], in_=WF_e[:])
            nc.sync.dma_start(out=BIAS0[:], in_=BIAS0_e[:])
            nc.sync.dma_start(out=BIASH[:], in_=BIASH_e[:])
            nc.sync.dma_start(out=OB1[:], in_=OB1_e[:])
            nc.sync.dma_start(out=OB2C[:], in_=OB2C_e[:])
            nc.sync.dma_start(out=MAGC[:], in_=MAGC_e[:])

            def whb(i, kh, fh):
                j = (i * 2 + kh) * 2 + fh
                return WH[:, j * 128:(j + 1) * 128]

            for t in range(T):
                c = cpool.tile([4, R], F32R, name="c")
                nc.sync.dma_start(out=c[:], in_=coordsT[:, t * R:(t + 1) * R])
                cf = cpool.tile([4, R], F32, name="cf")
                nc.sync.dma_start(out=cf[:], in_=coordsTF[:, t * R:(t + 1) * R])

                # ---- omega net ----
                phx = pz.tile([128, 2 * R], F32, name="pzz")
                phh = phx[0:64, 0:R]
                nc.tensor.matmul(phh, OW1[:], c[:], start=True, stop=True)
                h = work.tile([64, R], F32R, name="h")
                nc.scalar.activation(h[:], phh, A.Relu, bias=OB1[:, 0:1])
                puu = pz.tile([128, 2 * R], F32, name="pzz")
                nc.tensor.matmul(puu[:, 0:R], OW2R[:], h[:], start=True, stop=True)
                nc.tensor.matmul(puu[:, R:2 * R], OW2R[:], h[:], start=True, stop=True)
                sg = work.tile([128, 2 * R], F32, name="sg")
                nc.scalar.activation(sg[:], puu[:], A.Sigmoid, bias=OB2C[:, 0:1])
                om2 = work.tile([128, 2 * R], F32, name="om2")
                nc.vector.tensor_scalar(om2[:], sg[:], (MAX_O - MIN_O) * INV2PI,
                                        MIN_O * INV2PI, ALU.mult, ALU.add)
                om2r = work.tile([1, R], F32R, name="om2r")
                nc.vector.tensor_scalar(om2r[:], om2[0:1, 0:R], 0.0, None, ALU.add)

                STAGE = os.environ.get("KB_STAGE", "full")
                if STAGE == "omega":
                    ot = opool.tile([1, R], F32, name="ot")
                    nc.vector.tensor_scalar(ot[:], om2[0:1, 0:R], 1.0, None, ALU.mult)
                    nc.sync.dma_start(out=out_e[t * R:(t + 1) * R], in_=ot[0:1, :])
                    continue

                # ---- layer 0 (coords -> 256, K=4) ----
                xs0 = cpool.tile([4, R], F32, name="xs0")
                nc.vector.tensor_tensor(out=xs0[:], in0=cf[:], in1=om2[0:4, 0:R],
                                        op=ALU.mult)
                pzz = pz.tile([128, 2 * R], F32, name="pzz")
                for fh in range(2):
                    zs = pzz[:, fh * R:(fh + 1) * R]
                    nc.tensor.matmul(zs, W0[:, fh * 128:(fh + 1) * 128], xs0[:],
                                     start=True, stop=False)
                    nc.tensor.matmul(zs, BIAS0[0:1, fh * 128:(fh + 1) * 128], om2[0:1, 0:R],
                                     start=False, stop=True)
                y = ypool.tile([128, 2 * R], F32R, name="y")
                u = work.tile([128, 2 * R], F32, name="u")
                v = work.tile([128, 2 * R], F32, name="v")
                nc.vector.tensor_scalar(u[:], pzz[:], MAGIC, None, ALU.add)
                nc.vector.scalar_tensor_tensor(v[:], u[:], MAGIC, pzz[:],
                                               ALU.subtract, ALU.subtract)
                # Arctan is compile-time patched to Sin2pi; scale=-1 flips v=-frac
                nc.scalar.activation(y[:], v[:], A.Identity if os.environ.get("KB_IDENT") else A.Arctan, scale=-1.0)

                if STAGE == "l0":
                    ot = opool.tile([1, R], F32, name="ot")
                    nc.vector.tensor_scalar(ot[:], y[0:1, 0:R], 1.0, None, ALU.mult)
                    nc.sync.dma_start(out=out_e[t * R:(t + 1) * R], in_=ot[0:1, :])
                    continue

                # ---- hidden layers ----
                NH_RUN = int(os.environ.get("KB_NH", str(NHL)))
                for i in range(NH_RUN):
                    xs = ypool.tile([128, 2 * R], F32R, name="xs")
                    xs_eng = nc.vector if os.environ.get("KB_NOGPS") else nc.gpsimd
                    xs_eng.tensor_tensor(out=xs[:], in0=y[:], in1=om2[:],
                                         op=ALU.mult)
                    pzz = pz.tile([128, 2 * R], F32, name="pzz")
                    for fh in range(2):
                        zs = pzz[:, fh * R:(fh + 1) * R]
                        nc.tensor.matmul(zs, whb(i, 0, fh), xs[:, 0:R],
                                         start=True, stop=False)
                        nc.tensor.matmul(zs, whb(i, 1, fh), xs[:, R:2 * R],
                                         start=False, stop=False)
                        nc.tensor.matmul(zs,
                                         BIASH[0:1, (2 * i + fh) * 128:(2 * i + fh + 1) * 128],
                                         om2r[0:1, :], start=False, stop=True)
                    y = ypool.tile([128, 2 * R], F32R, name="y")
                    u = work.tile([128, 2 * R], F32, name="u")
                    v = work.tile([128, 2 * R], F32, name="v")
                    if i < 1:
                        nc.vector.tensor_scalar(u[:], pzz[:], MAGIC, None, ALU.add)
                    else:
                        nc.scalar.activation(u[:], pzz[:], A.Identity,
                                             bias=MAGC[:, 0:1])
                    nc.vector.scalar_tensor_tensor(v[:], u[:], MAGIC, pzz[:],
                                                   ALU.subtract, ALU.subtract)
                    nc.scalar.activation(y[:], v[:], A.Identity if os.environ.get("KB_IDENT") else A.Arctan, scale=-1.0)

                # ---- final layer (256 -> 1) ----
                pox = pz.tile([128, 2 * R], F32, name="pzz")
                poo = pox[0:1, 0:R]
                nc.tensor.matmul(poo, WF[:, 0:1], y[:, 0:R], start=True, stop=False)
                nc.tensor.matmul(poo, WF[:, 1:2], y[:, R:2 * R], start=False,
                                 stop=True)
                ot = opool.tile([1, R], F32, name="ot")
                nc.vector.tensor_scalar(ot[:], poo, 0.0, None, ALU.add)
                nc.sync.dma_start(out=out_e[t * R:(t + 1) * R], in_=ot[0:1, :])

    _split_multiwaits(nc, mybir)
    return nc


def _split_multiwaits(nc, mybir):
    """This walrus build accepts only ONE sync wait per instruction: splice
    extra waits onto single-wait same-engine NOPs placed immediately before
    the over-subscribed instruction."""
    ctr = 0
    for fn in nc.m.functions:
        for bb in fn.blocks:
            insts = list(bb.instructions)
            out = []
            changed = False
            for inst in insts:
                si = inst.sync_info
                waits = list(si.on_wait) if si and si.on_wait else []
                if len(waits) > 1:
                    changed = True
                    for w in waits[:-1]:
                        ctr += 1
                        nop = mybir.InstNoOp(
                            name=f"I-waitfix-{ctr}",
                            engine=inst.engine,
                            sync_info=mybir.SyncInfo(on_wait=[w], on_update=[]),
                        )
                        out.append(nop)
                    inst.sync_info = mybir.SyncInfo(
                        on_wait=[waits[-1]], on_update=list(si.on_update)
                    )
                out.append(inst)
            if changed:
                bb.instructions = out
    return nc


def _install_sin2pi_patch():
    """bass has no Sin2pi enum; emit Arctan and rewrite the BIR json at
    compile time.  sin2pi lives in the same ACT table set as sigmoid."""
    import concourse.bass2jax as b2j
    from concourse.bass_utils import compile_bir_kernel

    def patched(bir_json, tmpdir, neff_name="file.neff"):
        bir_json = bir_json.replace(b'"func":"Arctan"', b'"func":"Sin2pi"')
        return compile_bir_kernel(bir_json, tmpdir, neff_name)

    b2j.compile_bir_kernel = patched


def _prep_inputs(coords, ow1, ob1, ow2, ob2, w0, b0, wh, bh, wf, bf):
    coords = np.asarray(coords, np.float32)
    WH = np.empty((128, 16 * 128), np.float32)
    wh = np.asarray(wh, np.float32)
    for i in range(NHL):
        for kh in range(2):
            for fh in range(2):
                j = (i * 2 + kh) * 2 + fh
                WH[:, j * 128:(j + 1) * 128] = wh[i, kh * 128:(kh + 1) * 128,
                                                  fh * 128:(fh + 1) * 128]
    W0 = np.asarray(w0, np.float32)
    OW1 = np.asarray(ow1, np.float32)
    OW2R = np.tile(np.asarray(ow2, np.float32), (1, 128))
    wf = np.asarray(wf, np.float32)
    WF = np.stack([wf[0:128, 0], wf[128:256, 0]], axis=1)
    BIAS = np.empty((10, 128), np.float32)  # packed to [1, 1280] below
    b0 = np.asarray(b0, np.float32)
    bh = np.asarray(bh, np.float32)
    BIAS[0] = b0[0:128]
    BIAS[1] = b0[128:256]
    for i in range(NHL):
        for fh in range(2):
            BIAS[2 + 2 * i + fh] = bh[i, fh * 128:(fh + 1) * 128]
    OB1 = np.asarray(ob1, np.float32).reshape(64, 1)
    OB2C = np.full((128, 1), np.float32(np.asarray(ob2).reshape(-1)[0]), np.float32)
    shared = {"WH": WH, "W0": W0, "OW1": OW1, "OW2R": OW2R, "WF": WF,
              "BIAS0": BIAS[0:2].reshape(1, 256),
              "BIASH": BIAS[2:10].reshape(1, 8 * 128),
              "OB1": OB1, "OB2C": OB2C,
              "MAGC": np.full((128, 1), MAGIC, np.float32)}
    in_maps = []
    for cix in range(NC):
        shard = coords[cix * NCORE:(cix + 1) * NCORE]          # [NCORE, 4]
        m = dict(shared)
        m["coordsT"] = np.ascontiguousarray(shard.T)           # [4, NCORE]
        m["coordsTF"] = m["coordsT"]
        in_maps.append(m)
    return in_maps, np.float32(np.asarray(bf).reshape(-1)[0])


def kernel(coords, ow1, ob1, ow2, ob2, w0, b0, wh, bh, wf, bf, _trace=False):
    from concourse.bass_utils import run_bass_kernel_spmd

    _install_sin2pi_patch()
    if "nc" not in _CACHE:
        _CACHE["nc"] = _build()
    nc = _CACHE["nc"]
    in_maps, bf_v = _prep_inputs(coords, ow1, ob1, ow2, ob2, w0, b0,
                                 wh, bh, wf, bf)
    res = run_bass_kernel_spmd(nc, in_maps, core_ids=list(range(NC)),
                               trace=_trace)
    _CACHE["last_res"] = res
    outs = [np.asarray(res.results[i]["out"]).reshape(NCORE) for i in range(NC)]
    full = np.concatenate(outs) + bf_v
    return full.reshape(N, 1).astype(np.float32)


# revision 18
# speedup vs baseline: 2.7763x; 2.7763x over previous
"""AdaptiveSpectralDNA Trainium2 kernel: 8-core data-parallel SIREN MLP.

Feature-major activations (features on partitions, rows on free dim) so no
transposes are needed.  Hidden/final matmuls in fp16 (same 10-bit mantissa
as tf32/float32r, but 2-byte weights load fast); layer-0 and its large
+-300 rad arguments in fp32; the omega net in float32r.  Biases enter PSUM
exactly via K=2 fp16 hi/lo rank-1 matmuls against a constant ones vector.
sin(omega*(Wx+b)) = sin2pi(frac(t2)), t2 = (omega/2pi)*(Wx+b);
frac via the fp32 magic-number round, sin2pi via a BIR patch (bass has no
enum for it; it shares an ACT table set with sigmoid so no table switches).
"""
import os
import numpy as np

N = 524288
NHL = 4
NC = 8
NCORE = N // NC          # 65536 rows per core
R = 512                  # rows per tile (one PSUM bank of fp32)
T = NCORE // R           # 128 tiles
MAGIC = float(1.5 * 2**23)
INV2PI = float(1.0 / (2.0 * np.pi))
MIN_O, MAX_O = 10.0, 100.0

_CACHE = {}


def _build():
    import concourse.bass as bass
    import concourse.mybir as mybir
    from concourse.tile import TileContext

    F32 = mybir.dt.float32
    F32R = mybir.dt.float32r
    F16 = mybir.dt.float16
    A = mybir.ActivationFunctionType
    ALU = mybir.AluOpType

    nc = bass.Bass()
    coordsT = nc.declare_dram_parameter("coordsT", [4, NCORE], F32R, isOutput=False)
    coordsTF = nc.declare_dram_parameter("coordsTF", [4, NCORE], F32, isOutput=False)
    WH_e = nc.declare_dram_parameter("WH", [128, 16 * 128], F16, isOutput=False)
    W0_e = nc.declare_dram_parameter("W0", [4, 256], F32, isOutput=False)
    OW1_e = nc.declare_dram_parameter("OW1", [4, 64], F32R, isOutput=False)
    OW2R_e = nc.declare_dram_parameter("OW2R", [64, 128], F32R, isOutput=False)
    WF_e = nc.declare_dram_parameter("WF", [128, 2], F16, isOutput=False)
    BH2_e = nc.declare_dram_parameter("BH2", [2, 10 * 128], F16, isOutput=False)
    ONES2_e = nc.declare_dram_parameter("ONES2", [2, R], F16, isOutput=False)
    OB1_e = nc.declare_dram_parameter("OB1", [64, 1], F32, isOutput=False)
    OB2C_e = nc.declare_dram_parameter("OB2C", [128, 1], F32, isOutput=False)
    MAGC_e = nc.declare_dram_parameter("MAGC", [128, 1], F32, isOutput=False)
    BCOL_e = nc.declare_dram_parameter("BCOL", [128, 10], F32, isOutput=False)
    out_e = nc.declare_dram_parameter("out", [NCORE], F32, isOutput=True)

    with TileContext(nc) as tc:
        with (
            tc.tile_pool(name="wpool", bufs=1) as wpool,
            tc.tile_pool(name="cpool", bufs=6) as cpool,
            tc.tile_pool(name="work", bufs=5) as work,
            tc.tile_pool(name="uvt", bufs=5) as uvt,
            tc.tile_pool(name="ompool", bufs=6) as ompool,
            tc.tile_pool(name="ypool", bufs=9) as ypool,
            tc.tile_pool(name="opool", bufs=3) as opool,
            tc.tile_pool(name="pz", bufs=4, space="PSUM") as pz,
        ):
            WH = wpool.tile([128, 16 * 128], F16)
            W0 = wpool.tile([4, 256], F32)
            OW1 = wpool.tile([4, 64], F32R)
            OW2R = wpool.tile([64, 128], F32R)
            WF = wpool.tile([128, 2], F16)
            BH2 = wpool.tile([2, 10 * 128], F16)
            ONES2 = wpool.tile([2, R], F16)
            OB1 = wpool.tile([64, 1], F32)
            OB2C = wpool.tile([128, 1], F32)
            MAGC = wpool.tile([128, 1], F32)
            BCOL = wpool.tile([128, 10], F32)
            MAGBC = wpool.tile([128, 2 * R], F32)
            nc.gpsimd.memset(MAGBC[:], MAGIC)
            for dst, src in [(WH, WH_e), (W0, W0_e), (OW1, OW1_e),
                             (OW2R, OW2R_e), (WF, WF_e), (BH2, BH2_e),
                             (ONES2, ONES2_e), (OB1, OB1_e), (OB2C, OB2C_e),
                             (MAGC, MAGC_e), (BCOL, BCOL_e)]:
                nc.sync.dma_start(out=dst[:], in_=src[:])

            def whb(i, kh, fh):
                j = (i * 2 + kh) * 2 + fh
                return WH[:, j * 128:(j + 1) * 128]

            def emit_omega(st):
                t = st["t"]
                c = cpool.tile([4, R], F32R, name="c")
                nc.sync.dma_start(out=c[:], in_=coordsT[:, t * R:(t + 1) * R])
                cf = cpool.tile([4, R], F32, name="cf")
                nc.sync.dma_start(out=cf[:], in_=coordsTF[:, t * R:(t + 1) * R])
                st["cf"] = cf
                phx = pz.tile([128, 2 * R], F32, name="pzz")
                nc.tensor.matmul(phx[0:64, 0:R], OW1[:], c[:], start=True, stop=True)
                h = work.tile([64, R], F32R, name="h")
                nc.scalar.activation(h[:], phx[0:64, 0:R], A.Relu, bias=OB1[:, 0:1])
                nc.tensor.matmul(phx[:, R:2 * R], OW2R[:], h[:], start=True, stop=True)
                sg = work.tile([128, R], F32, name="sg")
                nc.scalar.activation(sg[:], phx[:, R:2 * R], A.Sigmoid,
                                     bias=OB2C[:, 0:1])
                om2 = ompool.tile([128, 2 * R], F32, name="om2")
                nc.vector.tensor_scalar(om2[:, 0:R], sg[:], (MAX_O - MIN_O) * INV2PI,
                                        MIN_O * INV2PI, ALU.mult, ALU.add)
                nc.vector.tensor_scalar(om2[:, R:2 * R], sg[:], (MAX_O - MIN_O) * INV2PI,
                                        MIN_O * INV2PI, ALU.mult, ALU.add)
                st["om2"] = om2

            def emit_stage_mms(sts, s):
                for st in sts:
                    st["pzz"] = pz.tile([128, 2 * R], F32, name="pzz")
                    for fh in range(2):
                        zs = st["pzz"][:, fh * R:(fh + 1) * R]
                        if s == 0:
                            nc.tensor.matmul(zs, W0[:, fh * 128:(fh + 1) * 128],
                                             st["cf"][:], start=True, stop=False)
                        else:
                            i = s - 1
                            nc.tensor.matmul(zs, whb(i, 0, fh), st["y"][:, 0:R],
                                             start=True, stop=False)
                            nc.tensor.matmul(zs, whb(i, 1, fh), st["y"][:, R:2 * R],
                                             start=False, stop=False)
                        j = 2 * s + fh
                        nc.tensor.matmul(zs, BH2[:, j * 128:(j + 1) * 128],
                                         ONES2[:], start=False, stop=True)

            def emit_stage_elem(st, s):
                pzz = st["pzz"]
                om2 = st["om2"]
                tt = uvt.tile([128, 2 * R], F32, name="tt")
                nc.vector.tensor_tensor(out=tt[:], in0=pzz[:], in1=om2[:],
                                        op=ALU.mult)
                u = uvt.tile([128, 2 * R], F32, name="u")
                if s in (0, 1, 2):
                    nc.scalar.activation(u[:], tt[:], A.Identity, bias=MAGC[:, 0:1])
                else:
                    nc.gpsimd.tensor_tensor(out=u[:], in0=tt[:], in1=MAGBC[:],
                                            op=ALU.add)
                v = uvt.tile([128, 2 * R], F32, name="v")
                nc.vector.scalar_tensor_tensor(v[:], u[:], MAGIC, tt[:],
                                               ALU.subtract, ALU.subtract)
                y2 = ypool.tile([128, 2 * R], F16, name="y")
                nc.scalar.activation(y2[:], v[:], A.Arctan, scale=-1.0)
                st["y"] = y2

            def emit_final_pair(stA, stB):
                pox = pz.tile([128, 2 * R], F32, name="pzz")
                for k, st in ((0, stA), (1, stB)):
                    poo = pox[0:1, k * R:(k + 1) * R]
                    y = st["y"]
                    nc.tensor.matmul(poo, WF[:, 0:1], y[:, 0:R],
                                     start=True, stop=False)
                    nc.tensor.matmul(poo, WF[:, 1:2], y[:, R:2 * R],
                                     start=False, stop=True)
                ot = opool.tile([1, 2 * R], F32, name="ot")
                nc.scalar.activation(ot[:], pox[0:1, :], A.Copy)
                tA = stA["t"]
                nc.sync.dma_start(out=out_e[tA * R:(tA + 2) * R], in_=ot[0:1, :])

            GW = int(os.environ.get("KB_GW", "6"))
            assert T % GW == 0, (T, GW)
            for tq in range(T // GW):
                sts = [{"t": GW * tq + k} for k in range(GW)]
                for st in sts:
                    emit_omega(st)
                for s in range(5):
                    emit_stage_mms(sts, s)
                    for st in sts:
                        emit_stage_elem(st, s)
                for k in range(0, GW, 2):
                    emit_final_pair(sts[k], sts[k + 1])

    _split_multiwaits(nc, mybir)
    return nc


def _split_multiwaits(nc, mybir):
    """This walrus build accepts only ONE sync wait per instruction: splice
    extra waits onto single-wait same-engine NOPs placed just before the
    over-subscribed instruction (engine streams are in-order)."""
    ctr = 0
    for fn in nc.m.functions:
        for bb in fn.blocks:
            insts = list(bb.instructions)
            out = []
            changed = False
            for inst in insts:
                si = inst.sync_info
                waits = list(si.on_wait) if si and si.on_wait else []
                if len(waits) > 1:
                    changed = True
                    for w in waits[:-1]:
                        ctr += 1
                        nop = mybir.InstNoOp(
                            name=f"I-waitfix-{ctr}",
                            engine=inst.engine,
                            sync_info=mybir.SyncInfo(on_wait=[w], on_update=[]),
                        )
                        out.append(nop)
                    inst.sync_info = mybir.SyncInfo(
                        on_wait=[waits[-1]], on_update=list(si.on_update)
                    )
                out.append(inst)
            if changed:
                bb.instructions = out
    return nc


def _install_sin2pi_patch():
    import concourse.bass2jax as b2j
    import concourse.bass_utils as bu
    from concourse.bass_utils import compile_bir_kernel

    def patched(bir_json, tmpdir, neff_name="file.neff"):
        bir_json = bir_json.replace(b'"func":"Arctan"', b'"func":"Sin2pi"')
        return compile_bir_kernel(bir_json, tmpdir, neff_name)

    b2j.compile_bir_kernel = patched
    if os.environ.get("KB_LDWOPT"):
        orig_run = bu.run_command

        def run_patched(argv, **kwargs):
            argv = ["--enable-ldw-opt=true" if a == "--enable-ldw-opt=false" else a
                    for a in argv]
            return orig_run(argv, **kwargs)

        if getattr(bu.run_command, "__name__", "") != "run_patched":
            bu.run_command = run_patched


def _hi_lo_f16(x):
    hi = x.astype(np.float16)
    lo = ((x - hi.astype(np.float32)) * 256.0).astype(np.float16)
    return hi, lo


def _prep_inputs(coords, ow1, ob1, ow2, ob2, w0, b0, wh, bh, wf, bf):
    coords = np.asarray(coords, np.float32)
    wh = np.asarray(wh, np.float32)
    WH = np.empty((128, 16 * 128), np.float16)
    for i in range(NHL):
        for kh in range(2):
            for fh in range(2):
                j = (i * 2 + kh) * 2 + fh
                WH[:, j * 128:(j + 1) * 128] = wh[i, kh * 128:(kh + 1) * 128,
                                                  fh * 128:(fh + 1) * 128].astype(np.float16)
    W0 = np.asarray(w0, np.float32)
    OW1 = np.asarray(ow1, np.float32)
    OW2R = np.tile(np.asarray(ow2, np.float32), (1, 128))
    wf = np.asarray(wf, np.float32)
    WF = np.stack([wf[0:128, 0], wf[128:256, 0]], axis=1).astype(np.float16)
    # biases: K=2 hi/lo fp16 rank-1 rows; lo scaled by 256 (fp16 normal range)
    ball = np.empty((10, 128), np.float32)
    b0 = np.asarray(b0, np.float32)
    bh = np.asarray(bh, np.float32)
    ball[0] = b0[0:128]
    ball[1] = b0[128:256]
    for i in range(NHL):
        for fh in range(2):
            ball[2 + 2 * i + fh] = bh[i, fh * 128:(fh + 1) * 128]
    BH2 = np.empty((2, 10 * 128), np.float16)
    for j in range(10):
        hi, lo = _hi_lo_f16(ball[j])
        BH2[0, j * 128:(j + 1) * 128] = hi
        BH2[1, j * 128:(j + 1) * 128] = lo
    ONES2 = np.empty((2, R), np.float16)
    ONES2[0] = 1.0
    ONES2[1] = 1.0 / 256.0
    OB1 = np.asarray(ob1, np.float32).reshape(64, 1)
    OB2C = np.full((128, 1), np.float32(np.asarray(ob2).reshape(-1)[0]), np.float32)
    shared = {"WH": WH, "W0": W0, "OW1": OW1, "OW2R": OW2R, "WF": WF,
              "BH2": BH2, "ONES2": ONES2, "OB1": OB1, "OB2C": OB2C,
              "MAGC": np.full((128, 1), MAGIC, np.float32),
              "BCOL": np.ascontiguousarray(ball.T)}
    in_maps = []
    for cix in range(NC):
        shard = coords[cix * NCORE:(cix + 1) * NCORE]
        m = dict(shared)
        m["coordsT"] = np.ascontiguousarray(shard.T)
        m["coordsTF"] = m["coordsT"]
        in_maps.append(m)
    return in_maps, np.float32(np.asarray(bf).reshape(-1)[0])


def kernel(coords, ow1, ob1, ow2, ob2, w0, b0, wh, bh, wf, bf, _trace=False):
    from concourse.bass_utils import run_bass_kernel_spmd

    _install_sin2pi_patch()
    if "nc" not in _CACHE:
        _CACHE["nc"] = _build()
    nc = _CACHE["nc"]
    in_maps, bf_v = _prep_inputs(coords, ow1, ob1, ow2, ob2, w0, b0,
                                 wh, bh, wf, bf)
    res = run_bass_kernel_spmd(nc, in_maps, core_ids=list(range(NC)),
                               trace=_trace)
    _CACHE["last_res"] = res
    outs = [np.asarray(res.results[i]["out"]).reshape(NCORE) for i in range(NC)]
    full = np.concatenate(outs) + bf_v
    return full.reshape(N, 1).astype(np.float32)


# revision 20
# speedup vs baseline: 2.8469x; 1.0254x over previous
"""AdaptiveSpectralDNA Trainium2 kernel: 8-core data-parallel SIREN MLP.

Feature-major activations (features on partitions, rows on free dim) so no
transposes are needed.  Hidden/final matmuls in fp16 (same 10-bit mantissa
as tf32/float32r, but 2-byte weights load fast); layer-0 and its large
+-300 rad arguments in fp32; the omega net in float32r.  Biases enter PSUM
exactly via K=2 fp16 hi/lo rank-1 matmuls against a constant ones vector.
sin(omega*(Wx+b)) = sin2pi(frac(t2)), t2 = (omega/2pi)*(Wx+b);
frac via the fp32 magic-number round, sin2pi via a BIR patch (bass has no
enum for it; it shares an ACT table set with sigmoid so no table switches).
"""
import os
import numpy as np

N = 524288
NHL = 4
NC = 8
NCORE = N // NC          # 65536 rows per core
R = 512                  # rows per tile (one PSUM bank of fp32)
T = NCORE // R           # 128 tiles
MAGIC = float(1.5 * 2**23)
INV2PI = float(1.0 / (2.0 * np.pi))
MIN_O, MAX_O = 10.0, 100.0

_CACHE = {}


def _build():
    import concourse.bass as bass
    import concourse.mybir as mybir
    from concourse.tile import TileContext

    F32 = mybir.dt.float32
    F32R = mybir.dt.float32r
    F16 = mybir.dt.float16
    A = mybir.ActivationFunctionType
    ALU = mybir.AluOpType

    nc = bass.Bass()
    coordsT = nc.declare_dram_parameter("coordsT", [4, NCORE], F32R, isOutput=False)
    coordsTF = nc.declare_dram_parameter("coordsTF", [4, NCORE], F32, isOutput=False)
    WH_e = nc.declare_dram_parameter("WH", [128, 16 * 128], F16, isOutput=False)
    W0_e = nc.declare_dram_parameter("W0", [4, 256], F32, isOutput=False)
    OW1_e = nc.declare_dram_parameter("OW1", [4, 64], F32R, isOutput=False)
    OW2R_e = nc.declare_dram_parameter("OW2R", [64, 128], F32R, isOutput=False)
    WF_e = nc.declare_dram_parameter("WF", [128, 2], F16, isOutput=False)
    BH2_e = nc.declare_dram_parameter("BH2", [2, 10 * 128], F16, isOutput=False)
    ONES2_e = nc.declare_dram_parameter("ONES2", [2, R], F16, isOutput=False)
    OB1_e = nc.declare_dram_parameter("OB1", [64, 1], F32, isOutput=False)
    OB2C_e = nc.declare_dram_parameter("OB2C", [128, 1], F32, isOutput=False)
    MAGC_e = nc.declare_dram_parameter("MAGC", [128, 1], F32, isOutput=False)
    BCOL_e = nc.declare_dram_parameter("BCOL", [128, 10], F32, isOutput=False)
    out_e = nc.declare_dram_parameter("out", [NCORE], F32, isOutput=True)

    with TileContext(nc) as tc:
        with (
            tc.tile_pool(name="wpool", bufs=1) as wpool,
            tc.tile_pool(name="cpool", bufs=8) as cpool,
            tc.tile_pool(name="work", bufs=8) as work,
            tc.tile_pool(name="uvt", bufs=5) as uvt,
            tc.tile_pool(name="ompool", bufs=10) as ompool,
            tc.tile_pool(name="ypool", bufs=10) as ypool,
            tc.tile_pool(name="opool", bufs=3) as opool,
            tc.tile_pool(name="pz", bufs=4, space="PSUM") as pz,
        ):
            WH = wpool.tile([128, 16 * 128], F16)
            W0 = wpool.tile([4, 256], F32)
            OW1 = wpool.tile([4, 64], F32R)
            OW2R = wpool.tile([64, 128], F32R)
            WF = wpool.tile([128, 2], F16)
            BH2 = wpool.tile([2, 10 * 128], F16)
            ONES2 = wpool.tile([2, R], F16)
            OB1 = wpool.tile([64, 1], F32)
            OB2C = wpool.tile([128, 1], F32)
            MAGC = wpool.tile([128, 1], F32)
            BCOL = wpool.tile([128, 10], F32)
            MAGBC = wpool.tile([128, 2 * R], F32)
            nc.gpsimd.memset(MAGBC[:], MAGIC)
            for dst, src in [(WH, WH_e), (W0, W0_e), (OW1, OW1_e),
                             (OW2R, OW2R_e), (WF, WF_e), (BH2, BH2_e),
                             (ONES2, ONES2_e), (OB1, OB1_e), (OB2C, OB2C_e),
                             (MAGC, MAGC_e), (BCOL, BCOL_e)]:
                nc.sync.dma_start(out=dst[:], in_=src[:])

            def whb(i, kh, fh):
                j = (i * 2 + kh) * 2 + fh
                return WH[:, j * 128:(j + 1) * 128]

            def emit_omega(st):
                t = st["t"]
                c = cpool.tile([4, R], F32R, name="c")
                nc.sync.dma_start(out=c[:], in_=coordsT[:, t * R:(t + 1) * R])
                cf = cpool.tile([4, R], F32, name="cf")
                nc.sync.dma_start(out=cf[:], in_=coordsTF[:, t * R:(t + 1) * R])
                st["cf"] = cf
                phx = pz.tile([128, 2 * R], F32, name="pzz")
                nc.tensor.matmul(phx[0:64, 0:R], OW1[:], c[:], start=True, stop=True)
                h = work.tile([64, R], F32R, name="h")
                nc.scalar.activation(h[:], phx[0:64, 0:R], A.Relu, bias=OB1[:, 0:1])
                nc.tensor.matmul(phx[:, R:2 * R], OW2R[:], h[:], start=True, stop=True)
                sg = work.tile([128, R], F32, name="sg")
                nc.scalar.activation(sg[:], phx[:, R:2 * R], A.Sigmoid,
                                     bias=OB2C[:, 0:1])
                om2 = ompool.tile([128, R], F32, name="om2")
                nc.vector.tensor_scalar(om2[:], sg[:], (MAX_O - MIN_O) * INV2PI,
                                        MIN_O * INV2PI, ALU.mult, ALU.add)
                st["om2"] = om2

            def emit_stage_mms(sts, s):
                for st in sts:
                    st["pzz"] = pz.tile([128, 2 * R], F32, name="pzz")
                    for fh in range(2):
                        zs = st["pzz"][:, fh * R:(fh + 1) * R]
                        if s == 0:
                            nc.tensor.matmul(zs, W0[:, fh * 128:(fh + 1) * 128],
                                             st["cf"][:], start=True, stop=False)
                        else:
                            i = s - 1
                            nc.tensor.matmul(zs, whb(i, 0, fh), st["y"][:, 0:R],
                                             start=True, stop=False)
                            nc.tensor.matmul(zs, whb(i, 1, fh), st["y"][:, R:2 * R],
                                             start=False, stop=False)
                        j = 2 * s + fh
                        nc.tensor.matmul(zs, BH2[:, j * 128:(j + 1) * 128],
                                         ONES2[:], start=False, stop=True)

            def emit_stage_elem(st, s):
                pzz = st["pzz"]
                om2 = st["om2"]
                tt = uvt.tile([128, 2 * R], F32, name="tt")
                import dataclasses as _dc
                om2f = om2[:, 0:R]
                om2rep = _dc.replace(om2f, ap=[om2f.ap[0], [0, 2], [1, R]])
                nc.vector.tensor_tensor(out=tt[:], in0=pzz[:], in1=om2rep,
                                        op=ALU.mult)
                u = uvt.tile([128, 2 * R], F32, name="u")
                if s in (0, 1, 2):
                    nc.scalar.activation(u[:], tt[:], A.Identity, bias=MAGC[:, 0:1])
                else:
                    nc.gpsimd.tensor_tensor(out=u[:], in0=tt[:], in1=MAGBC[:],
                                            op=ALU.add)
                v = uvt.tile([128, 2 * R], F32, name="v")
                nc.vector.scalar_tensor_tensor(v[:], u[:], MAGIC, tt[:],
                                               ALU.subtract, ALU.subtract)
                y2 = ypool.tile([128, 2 * R], F16, name="y")
                nc.scalar.activation(y2[:], v[:], A.Arctan, scale=-1.0)
                st["y"] = y2

            def emit_final_pair(stA, stB):
                pox = pz.tile([128, 2 * R], F32, name="pzz")
                for k, st in ((0, stA), (1, stB)):
                    poo = pox[0:1, k * R:(k + 1) * R]
                    y = st["y"]
                    nc.tensor.matmul(poo, WF[:, 0:1], y[:, 0:R],
                                     start=True, stop=False)
                    nc.tensor.matmul(poo, WF[:, 1:2], y[:, R:2 * R],
                                     start=False, stop=True)
                ot = opool.tile([1, 2 * R], F32, name="ot")
                nc.scalar.activation(ot[:], pox[0:1, :], A.Copy)
                tA = stA["t"]
                nc.sync.dma_start(out=out_e[tA * R:(tA + 2) * R], in_=ot[0:1, :])

            GW = int(os.environ.get("KB_GW", "6"))
            assert T % GW == 0, (T, GW)
            for tq in range(T // GW):
                sts = [{"t": GW * tq + k} for k in range(GW)]
                for st in sts:
                    emit_omega(st)
                for s in range(5):
                    emit_stage_mms(sts, s)
                    for st in sts:
                        emit_stage_elem(st, s)
                for k in range(0, GW, 2):
                    emit_final_pair(sts[k], sts[k + 1])

    _split_multiwaits(nc, mybir)
    return nc


def _split_multiwaits(nc, mybir):
    """This walrus build accepts only ONE sync wait per instruction: splice
    extra waits onto single-wait same-engine NOPs placed just before the
    over-subscribed instruction (engine streams are in-order)."""
    ctr = 0
    for fn in nc.m.functions:
        for bb in fn.blocks:
            insts = list(bb.instructions)
            out = []
            changed = False
            for inst in insts:
                si = inst.sync_info
                waits = list(si.on_wait) if si and si.on_wait else []
                if len(waits) > 1:
                    changed = True
                    for w in waits[:-1]:
                        ctr += 1
                        nop = mybir.InstNoOp(
                            name=f"I-waitfix-{ctr}",
                            engine=inst.engine,
                            sync_info=mybir.SyncInfo(on_wait=[w], on_update=[]),
                        )
                        out.append(nop)
                    inst.sync_info = mybir.SyncInfo(
                        on_wait=[waits[-1]], on_update=list(si.on_update)
                    )
                out.append(inst)
            if changed:
                bb.instructions = out
    return nc


def _install_sin2pi_patch():
    import concourse.bass2jax as b2j
    import concourse.bass_utils as bu
    from concourse.bass_utils import compile_bir_kernel

    def patched(bir_json, tmpdir, neff_name="file.neff"):
        bir_json = bir_json.replace(b'"func":"Arctan"', b'"func":"Sin2pi"')
        return compile_bir_kernel(bir_json, tmpdir, neff_name)

    b2j.compile_bir_kernel = patched
    if os.environ.get("KB_LDWOPT"):
        orig_run = bu.run_command

        def run_patched(argv, **kwargs):
            argv = ["--enable-ldw-opt=true" if a == "--enable-ldw-opt=false" else a
                    for a in argv]
            return orig_run(argv, **kwargs)

        if getattr(bu.run_command, "__name__", "") != "run_patched":
            bu.run_command = run_patched


def _hi_lo_f16(x):
    hi = x.astype(np.float16)
    lo = ((x - hi.astype(np.float32)) * 256.0).astype(np.float16)
    return hi, lo


def _prep_inputs(coords, ow1, ob1, ow2, ob2, w0, b0, wh, bh, wf, bf):
    coords = np.asarray(coords, np.float32)
    wh = np.asarray(wh, np.float32)
    WH = np.empty((128, 16 * 128), np.float16)
    for i in range(NHL):
        for kh in range(2):
            for fh in range(2):
                j = (i * 2 + kh) * 2 + fh
                WH[:, j * 128:(j + 1) * 128] = wh[i, kh * 128:(kh + 1) * 128,
                                                  fh * 128:(fh + 1) * 128].astype(np.float16)
    W0 = np.asarray(w0, np.float32)
    OW1 = np.asarray(ow1, np.float32)
    OW2R = np.tile(np.asarray(ow2, np.float32), (1, 128))
    wf = np.asarray(wf, np.float32)
    WF = np.stack([wf[0:128, 0], wf[128:256, 0]], axis=1).astype(np.float16)
    # biases: K=2 hi/lo fp16 rank-1 rows; lo scaled by 256 (fp16 normal range)
    ball = np.empty((10, 128), np.float32)
    b0 = np.asarray(b0, np.float32)
    bh = np.asarray(bh, np.float32)
    ball[0] = b0[0:128]
    ball[1] = b0[128:256]
    for i in range(NHL):
        for fh in range(2):
            ball[2 + 2 * i + fh] = bh[i, fh * 128:(fh + 1) * 128]
    BH2 = np.empty((2, 10 * 128), np.float16)
    for j in range(10):
        hi, lo = _hi_lo_f16(ball[j])
        BH2[0, j * 128:(j + 1) * 128] = hi
        BH2[1, j * 128:(j + 1) * 128] = lo
    ONES2 = np.empty((2, R), np.float16)
    ONES2[0] = 1.0
    ONES2[1] = 1.0 / 256.0
    OB1 = np.asarray(ob1, np.float32).reshape(64, 1)
    OB2C = np.full((128, 1), np.float32(np.asarray(ob2).reshape(-1)[0]), np.float32)
    shared = {"WH": WH, "W0": W0, "OW1": OW1, "OW2R": OW2R, "WF": WF,
              "BH2": BH2, "ONES2": ONES2, "OB1": OB1, "OB2C": OB2C,
              "MAGC": np.full((128, 1), MAGIC, np.float32),
              "BCOL": np.ascontiguousarray(ball.T)}
    in_maps = []
    for cix in range(NC):
        shard = coords[cix * NCORE:(cix + 1) * NCORE]
        m = dict(shared)
        m["coordsT"] = np.ascontiguousarray(shard.T)
        m["coordsTF"] = m["coordsT"]
        in_maps.append(m)
    return in_maps, np.float32(np.asarray(bf).reshape(-1)[0])


def kernel(coords, ow1, ob1, ow2, ob2, w0, b0, wh, bh, wf, bf, _trace=False):
    from concourse.bass_utils import run_bass_kernel_spmd

    _install_sin2pi_patch()
    if "nc" not in _CACHE:
        _CACHE["nc"] = _build()
    nc = _CACHE["nc"]
    in_maps, bf_v = _prep_inputs(coords, ow1, ob1, ow2, ob2, w0, b0,
                                 wh, bh, wf, bf)
    res = run_bass_kernel_spmd(nc, in_maps, core_ids=list(range(NC)),
                               trace=_trace)
    _CACHE["last_res"] = res
    outs = [np.asarray(res.results[i]["out"]).reshape(NCORE) for i in range(NC)]
    full = np.concatenate(outs) + bf_v
    return full.reshape(N, 1).astype(np.float32)
